# revision 10
# baseline (speedup 1.0000x reference)
"""Multi-head attention (B=2, S=2048, H=1024, 16 heads) on 8 TRN2 NeuronCores.

Sharding: tensor-parallel over heads x data-parallel over batch.
core = b * 4 + g handles batch b and head-group g (4 heads, 256 channels).

All matmuls are bf16 (fp8 anywhere in the attention path costs ~1:1 of its
quantization noise on the output rms - the output magnitude shrinks through
the softmax averaging exactly as fast as incoherent errors do - and blows
the 2e-2 budget).  Device-side dataflow (fp32 PSUM accumulation):

  x_t    [H, S]      = hidden[b].T                  (host-transposed)
  qk_T   [512, S]    = Wqk_g x_t                    (chunks: qA qB kA kB)
  v      [S, 256]    = x w_v.T                      (lhsT = x_t chunks)
  st     [128k, 1024]= scores, 2 heads row-split-packed per kt tile
  pt     = exp(st * scale + mask[k]), split across TWO engines:
             ACT:  spline Exp  (most tiles)
             DVE:  Schraudolph int16 bits = round(st*(scale*A16) + s2);
                   the i16 value IS the bf16 bit pattern (2% rms, capped
                   to a few tiles per window to protect accuracy)
  av     [128, 512]  = v_aug^T pt per head; rows 0:64 unnormalized out.T,
                       rows 64:128 = Z (v_aug cols 64:128 are ones)
  attn_T [256, S]    = av[:64] * reciprocal(av[64:128])
  out_t  [H, S]      = Wo_g^T-contracted partial output (transposed)

Orchestration (the v1 baseline lost ~65us of engine idle to these):
  - x is DMAed per 512-token window and the k chunks + v project first, so
    the PE starts ~4us in instead of ~28us; dummy warm-up matmuls keep the
    PE clock at 2.4GHz through the DMA wait.
  - pair B's projection fills pair A's attention windows; out_proj(w) is
    interleaved into window w+1's kt loop so the exp engines never stall
    behind it; only out_proj(last) remains as a tail.
  - input DMAs issue from sync/gpsimd only; Scalar stays clear for exp.

Host sums the 4 group partials per batch, transposes back, and adds the
exact bias corrections: b_out plus w_out @ b_v.
"""

import math

import numpy as np

import concourse.tile as tile
from concourse import bacc, mybir
from concourse.bass_utils import run_bass_kernel_spmd

B, S, H = 2, 2048, 1024
NH, HD = 16, 64
NCORES = 8
NGROUP = 4              # head groups = cores per batch
HPG = NH // NGROUP      # 4 heads per group
DG = HPG * HD           # 256 channels per group
P = 128
SCALE = float(HD) ** -0.5

FP32 = mybir.dt.float32
BF16 = mybir.dt.bfloat16
I16 = mybir.dt.int16

S_TILES = S // P        # 16 key/token tiles
HC = H // P             # 8 contraction chunks over H
QKR = 2 * DG            # 512 q+k rows
QKC = QKR // P          # 4 chunks of qk rows
QT = 1024               # scores tile: 2 heads x 512 q
WIN = 512               # q window
NWIN = S // WIN         # 4

A16 = 128.0 / math.log(2.0)
B16 = 16250.5           # Schraudolph bias (RNE f32->i16, bf16 bit pattern)

# kt tiles routed to the DVE Schraudolph exp (per window); keep small:
# each tile carries ~2% rms error vs ACT's exact exp.
DVE_KTS = (2, 5, 8, 11, 14)

_NC_CACHE = {}
LAST_RESULT = None      # BassKernelResults of the most recent run (for test.py)


def _body(tc, x_t, wqk_t, wv_t, wo_t, act_bias, mask_info, bias_zero, out_t):
    nc = tc.nc
    mask_uniform, dve_s2_imm, no_dve = mask_info
    with (
        tc.tile_pool(name="const", bufs=1) as const,
        tc.tile_pool(name="big", bufs=1) as big,
        tc.tile_pool(name="pt_pool", bufs=4) as pt_pool,
        tc.tile_pool(name="rz_pool", bufs=2) as rz_pool,
        tc.tile_pool(name="osb_pool", bufs=2) as osb_pool,
        tc.tile_pool(name="stp", bufs=2, space="PSUM") as stp,
        tc.tile_pool(name="avps", bufs=3, space="PSUM") as avps,
        tc.tile_pool(name="iop", bufs=1, space="PSUM") as iop,
    ):
        # ---------- PE warm-up: keep HAM busy during the input DMA ----------
        wu_sb = const.tile([P, 128], BF16, name="wu_sb")
        nc.vector.memset(wu_sb[:], 0.0)
        wu_ps = iop.tile([P, 128], FP32, name="wu_ps", tag="io")
        for _ in range(24):
            nc.tensor.matmul(wu_ps[:], lhsT=wu_sb[:], rhs=wu_sb[:],
                             start=True, stop=True)

        # ---------- input DMAs, ordered by first use ----------
        x_sb = big.tile([P, HC, S], BF16, name="x_sb")
        x_r = x_t.rearrange("(c p) s -> p c s", p=P)
        wqk_sb = const.tile([P, HC, QKR], BF16, name="wqk_sb")
        wqk_r = wqk_t.rearrange("(c p) r -> p c r", p=P)
        nc.sync.dma_start(wqk_sb[:], wqk_r[:])
        nc.gpsimd.dma_start(x_sb[:, 0:4, 0:WIN], x_r[:, 0:4, 0:WIN])
        nc.scalar.dma_start(x_sb[:, 4:8, 0:WIN], x_r[:, 4:8, 0:WIN])
        nc.gpsimd.dma_start(x_sb[:, :, WIN:2 * WIN], x_r[:, :, WIN:2 * WIN])
        nc.sync.dma_start(x_sb[:, :, 2 * WIN:3 * WIN], x_r[:, :, 2 * WIN:3 * WIN])
        nc.gpsimd.dma_start(x_sb[:, :, 3 * WIN:S], x_r[:, :, 3 * WIN:S])
        wv_sb = const.tile([P, HC, DG], BF16, name="wv_sb")
        nc.gpsimd.dma_start(wv_sb[:], wv_t.rearrange("(c p) r -> p c r", p=P))
        wo_sb = const.tile([P, DG // P, H], BF16, name="wo_sb")
        nc.sync.dma_start(wo_sb[:], wo_t.rearrange("(c p) r -> p c r", p=P))
        ab_sb = const.tile([P, S_TILES], FP32, name="ab_sb")
        nc.gpsimd.dma_start(ab_sb[:], act_bias)
        if not mask_uniform:
            db_sb = const.tile([P, S_TILES], FP32, name="db_sb")
            nc.gpsimd.dma_start(db_sb[:], tc.extra["dve_bias"])
        if not bias_zero:
            bqk_sb = const.tile([P, QKC], FP32, name="bqk_sb")
            nc.gpsimd.dma_start(bqk_sb[:], tc.extra["bqk"])

        qk_sb = big.tile([P, QKC, S], BF16, name="qk_sb")
        # v_aug per (tile, head): [v (64 cols) | ones (64 cols)]
        v_sb = big.tile([P, S_TILES, HPG, 2 * HD], BF16, name="v_sb")
        attn_sb = big.tile([P, DG // P, S], BF16, name="attn_sb")

        ones_sb = const.tile([P, HPG, HD], BF16, name="ones_sb")
        nc.vector.memset(ones_sb[:], 1.0)
        for tt in range(S_TILES):
            nc.vector.tensor_copy(v_sb[:, tt, :, HD:2 * HD], ones_sb[:])

        # ---------- qkv projection ----------
        def qk_group(rc, i, pool):
            qk_ps = pool.tile([P, 512], FP32, name="qk_ps",
                              tag="av" if pool is avps else "io")
            for c in range(HC):
                nc.tensor.matmul(
                    qk_ps[:],
                    lhsT=wqk_sb[:, c, rc * P:(rc + 1) * P],
                    rhs=x_sb[:, c, i * WIN:(i + 1) * WIN],
                    start=(c == 0), stop=(c == HC - 1),
                )
            dst = qk_sb[:, rc, i * WIN:(i + 1) * WIN]
            if bias_zero:
                nc.vector.tensor_copy(dst, qk_ps[:])
            else:
                nc.vector.tensor_scalar_add(dst, qk_ps[:], bqk_sb[:, rc:rc + 1])

        def v_group(tp):
            # token tiles (2tp, 2tp+1) side by side in one psum slot
            v_ps = avps.tile([P, 512], FP32, name="v_ps", tag="av")
            for half in range(2):
                tt = 2 * tp + half
                for c in range(HC):
                    nc.tensor.matmul(
                        v_ps[:, half * DG:(half + 1) * DG],
                        lhsT=x_sb[:, c, tt * P:(tt + 1) * P],
                        rhs=wv_sb[:, c, :],
                        start=(c == 0), stop=(c == HC - 1),
                    )
            nc.vector.tensor_copy(
                v_sb[:, 2 * tp:2 * tp + 2, :, 0:HD],
                v_ps[:].rearrange("p (t h d) -> p t h d", t=2, d=HD),
            )

        # k chunks + v first (attention needs every token of k/v), then q(0)
        qk_group(2, 0, avps)
        for i in range(1, NWIN):
            qk_group(2, i, avps)
            v_group(2 * (i - 1))
            v_group(2 * (i - 1) + 1)
        v_group(6)
        v_group(7)
        qk_group(0, 0, avps)

        # ---------- attention ----------
        o_r = out_t.rearrange("(c p) s -> p c s", p=P)
        dve_s1 = SCALE * A16

        def out_proj_chunks(w, jjs, o_sb):
            qlo = w * WIN
            for jj in jjs:
                o_ps = iop.tile([P, 512], FP32, name="o_ps", tag="io")
                for kc in range(DG // P):
                    nc.tensor.matmul(
                        o_ps[:],
                        lhsT=wo_sb[:, kc, jj * P:(jj + 1) * P],
                        rhs=attn_sb[:, kc, qlo:qlo + WIN],
                        start=(kc == 0), stop=(kc == DG // P - 1),
                    )
                if jj % 2 == 0:
                    nc.scalar.copy(o_sb[:, jj, :], o_ps[:])
                else:
                    nc.vector.tensor_copy(o_sb[:, jj, :], o_ps[:])

        def attn_window(qc, w, hooks):
            """One 512-q window for head pair qc. hooks: {kt: [fn, ...]}"""
            qlo = w * WIN
            av = [avps.tile([P, 512], FP32, name=f"av{h}", tag="av")
                  for h in range(2)]
            for kt in range(S_TILES):
                st = stp.tile([P, QT], FP32, name="st", tag="st")
                for half in range(2):
                    off = half * HD
                    nc.tensor.matmul(
                        st[:, half * 512:(half + 1) * 512],
                        lhsT=qk_sb[off:off + HD, 2 + qc, kt * P:(kt + 1) * P],
                        rhs=qk_sb[off:off + HD, qc, qlo:qlo + WIN],
                        start=True, stop=True,
                    )
                pt = pt_pool.tile([P, QT], BF16, name="pt", tag="pt")
                if kt in DVE_KTS and not no_dve:
                    s2 = dve_s2_imm if mask_uniform else db_sb[:, kt:kt + 1]
                    nc.vector.tensor_scalar(
                        pt[:].bitcast(I16), st[:],
                        dve_s1, s2,
                        mybir.AluOpType.mult, mybir.AluOpType.add,
                    )
                else:
                    nc.scalar.activation(
                        pt[:], st[:],
                        mybir.ActivationFunctionType.Exp,
                        bias=ab_sb[:, kt:kt + 1],
                        scale=SCALE,
                    )
                for h in range(2):
                    nc.tensor.matmul(
                        av[h][:],
                        lhsT=v_sb[:, kt, 2 * qc + h, :],
                        rhs=pt[:, h * 512:(h + 1) * 512],
                        start=(kt == 0), stop=(kt == S_TILES - 1),
                    )
                for fn in hooks.get(kt, ()):
                    fn()
            zc = rz_pool.tile([HD, QT], FP32, name="zc", tag="zc")
            for h in range(2):
                nc.vector.tensor_copy(
                    zc[:, h * 512:(h + 1) * 512], av[h][HD:2 * HD, :])
            rz = rz_pool.tile([HD, QT], FP32, name="rz", tag="rz")
            nc.vector.reciprocal_approx_fast(rz[:], zc[:])
            for h in range(2):
                nc.vector.tensor_mul(
                    attn_sb[h * HD:(h + 1) * HD, qc, qlo:qlo + WIN],
                    av[h][0:HD, :],
                    rz[:, h * 512:(h + 1) * 512],
                )

        # pair A windows: interleave pair B's projection (+ pair A's next q)
        for w in range(NWIN):
            hooks = {3: [lambda w=w: qk_group(1, w, iop)],
                     9: [lambda w=w: qk_group(3, w, iop)]}
            if w < NWIN - 1:
                hooks[13] = [lambda w=w: qk_group(0, w + 1, iop)]
            attn_window(0, w, hooks)

        # pair B windows: interleave out-proj of the previous window
        for w in range(NWIN):
            hooks = {}
            if w > 0:
                pw = w - 1
                o_sb = osb_pool.tile([P, HC, 512], BF16, name="o_sb", tag="osb")
                hooks = {
                    2: [lambda pw=pw, o=o_sb: out_proj_chunks(pw, (0, 1), o)],
                    6: [lambda pw=pw, o=o_sb: out_proj_chunks(pw, (2, 3), o)],
                    10: [lambda pw=pw, o=o_sb: out_proj_chunks(pw, (4, 5), o)],
                    14: [lambda pw=pw, o=o_sb: out_proj_chunks(pw, (6, 7), o),
                         lambda pw=pw, o=o_sb: nc.sync.dma_start(
                             o_r[:, :, pw * WIN:(pw + 1) * WIN], o[:])],
                }
            attn_window(1, w, hooks)
        # tail: last window's out-proj, DMA pipelined per 2 chunks
        o_sb = osb_pool.tile([P, HC, 512], BF16, name="o_sb", tag="osb")
        qlo = (NWIN - 1) * WIN
        for j0 in range(0, HC, 2):
            out_proj_chunks(NWIN - 1, (j0, j0 + 1), o_sb)
            nc.sync.dma_start(o_r[:, j0:j0 + 2, qlo:S], o_sb[:, j0:j0 + 2, :])


def _build(mask_uniform, dve_s2_imm, bias_zero, no_dve):
    nc = bacc.Bacc(
        "TRN2",
        target_bir_lowering=False,
        debug=False,
        enable_asserts=True,
        num_devices=NCORES,
    )
    x_t = nc.dram_tensor("x_t", [H, S], BF16, kind="ExternalInput").ap()
    wqk_t = nc.dram_tensor("wqk_t", [H, QKR], BF16, kind="ExternalInput").ap()
    wv_t = nc.dram_tensor("wv_t", [H, DG], BF16, kind="ExternalInput").ap()
    wo_t = nc.dram_tensor("wo_t", [DG, H], BF16, kind="ExternalInput").ap()
    act_bias = nc.dram_tensor("act_bias", [P, S_TILES], FP32,
                              kind="ExternalInput").ap()
    out_t = nc.dram_tensor("out_t", [H, S], BF16, kind="ExternalOutput").ap()

    with tile.TileContext(nc) as tc:
        tc.extra = {}
        if not mask_uniform:
            tc.extra["dve_bias"] = nc.dram_tensor(
                "dve_bias", [P, S_TILES], FP32, kind="ExternalInput").ap()
        if not bias_zero:
            tc.extra["bqk"] = nc.dram_tensor(
                "bqk", [P, QKC], FP32, kind="ExternalInput").ap()
        _body(tc, x_t, wqk_t, wv_t, wo_t, act_bias,
              (mask_uniform, dve_s2_imm, no_dve), bias_zero, out_t)
    nc.compile()
    return nc


def _get_nc(mask_uniform=True, dve_s2_imm=None, bias_zero=True):
    import os
    if dve_s2_imm is None:
        dve_s2_imm = B16
    no_dve = bool(int(os.environ.get("KERNEL_NO_DVE_EXP", "0")))
    key = (mask_uniform, dve_s2_imm, bias_zero, no_dve)
    if key not in _NC_CACHE:
        _NC_CACHE[key] = _build(mask_uniform, dve_s2_imm, bias_zero, no_dve)
    return _NC_CACHE[key]


def _variant(attention_mask, b_qkv):
    m0 = float(np.asarray(attention_mask).reshape(-1)[0])
    mask_uniform = bool(np.all(attention_mask == m0))
    dve_s2_imm = (B16 + A16 * m0) if mask_uniform else None
    bias_zero = bool(np.all(b_qkv == 0.0))
    return mask_uniform, dve_s2_imm, bias_zero


def make_in_maps(hidden_states, attention_mask, w_qkv, b_qkv, w_out):
    import ml_dtypes

    bf16 = ml_dtypes.bfloat16
    mask_uniform, _, bias_zero = _variant(attention_mask, b_qkv)
    in_maps = []
    for core in range(NCORES):
        b, g = divmod(core, NGROUP)
        wq = w_qkv[0 * H + g * DG:0 * H + (g + 1) * DG]
        wk = w_qkv[1 * H + g * DG:1 * H + (g + 1) * DG]
        wv = w_qkv[2 * H + g * DG:2 * H + (g + 1) * DG]
        mask = np.asarray(attention_mask[b], np.float32)
        ab = mask.reshape(S_TILES, P).T              # [P, S_TILES]
        m = {
            "x_t": np.ascontiguousarray(hidden_states[b].T).astype(bf16),
            "wqk_t": np.ascontiguousarray(
                np.concatenate([wq, wk], 0).T).astype(bf16),
            "wv_t": np.ascontiguousarray(wv.T).astype(bf16),
            "wo_t": np.ascontiguousarray(
                w_out[:, g * DG:(g + 1) * DG].T).astype(bf16),
            "act_bias": np.ascontiguousarray(ab, dtype=np.float32),
        }
        if not mask_uniform:
            m["dve_bias"] = np.ascontiguousarray(
                A16 * ab + B16, dtype=np.float32)
        if not bias_zero:
            m["bqk"] = np.ascontiguousarray(
                np.concatenate([b_qkv[g * DG:(g + 1) * DG],
                                b_qkv[H + g * DG:H + (g + 1) * DG]])
                .reshape(QKC, P).T, dtype=np.float32)
        in_maps.append(m)
    return in_maps


def kernel(hidden_states, attention_mask, w_qkv, b_qkv, w_out, b_out):
    global LAST_RESULT
    hidden_states = np.asarray(hidden_states, dtype=np.float32)
    attention_mask = np.asarray(attention_mask, dtype=np.float32)
    w_qkv = np.asarray(w_qkv, dtype=np.float32)
    b_qkv = np.asarray(b_qkv, dtype=np.float32)
    w_out = np.asarray(w_out, dtype=np.float32)
    b_out = np.asarray(b_out, dtype=np.float32)

    mask_uniform, dve_s2_imm, bias_zero = _variant(attention_mask, b_qkv)
    nc = _get_nc(mask_uniform, dve_s2_imm, bias_zero)
    in_maps = make_in_maps(hidden_states, attention_mask, w_qkv, b_qkv, w_out)

    import os
    trace = bool(int(os.environ.get("KERNEL_TRACE", "0")))
    res = run_bass_kernel_spmd(
        nc, in_maps, core_ids=list(range(NCORES)), trace=trace,
    )
    LAST_RESULT = res

    out = np.zeros((B, S, H), np.float32)
    vbias = w_out @ b_qkv[2 * H:]          # exact v-bias correction
    for b in range(B):
        acc = res.results[b * NGROUP + 0]["out_t"].astype(np.float32)
        for g in range(1, NGROUP):
            acc = acc + res.results[b * NGROUP + g]["out_t"].astype(np.float32)
        out[b] = acc.T + b_out + vbias
    return out


# revision 11
# speedup vs baseline: 1.0502x; 1.0502x over previous
"""Multi-head attention (B=2, S=2048, H=1024, 16 heads) on 8 TRN2 NeuronCores.

Sharding: tensor-parallel over heads x data-parallel over batch.
core = b * 4 + g handles batch b and head-group g (4 heads, 256 channels).

All matmuls are bf16 (fp8 anywhere in the attention path costs ~1:1 of its
quantization noise on the output rms - the output magnitude shrinks through
the softmax averaging exactly as fast as incoherent errors do - and blows
the 2e-2 budget).  Device-side dataflow (fp32 PSUM accumulation):

  x_t    [H, S]      = hidden[b].T                  (host-transposed)
  qk_T   [512, S]    = Wqk_g x_t                    (chunks: qA qB kA kB)
  v      [S, 256]    = x w_v.T                      (lhsT = x_t chunks)
  st     [128k, 1024]= scores, 2 heads row-split-packed per kt tile
  pt     = exp(st * scale + mask[k]), split across TWO engines:
             ACT:  spline Exp  (most tiles)
             DVE:  Schraudolph int16 bits = round(st*(scale*A16) + s2);
                   the i16 value IS the bf16 bit pattern (2% rms, capped
                   to a few tiles per window to protect accuracy)
  av     [128, 512]  = v_aug^T pt per head; rows 0:64 unnormalized out.T,
                       rows 64:128 = Z (v_aug cols 64:128 are ones)
  attn_T [256, S]    = av[:64] * reciprocal(av[64:128])
  out_t  [H, S]      = Wo_g^T-contracted partial output (transposed)

Orchestration (the v1 baseline lost ~65us of engine idle to these):
  - x is DMAed per 512-token window and the k chunks + v project first, so
    the PE starts ~4us in instead of ~28us; dummy warm-up matmuls keep the
    PE clock at 2.4GHz through the DMA wait.
  - pair B's projection fills pair A's attention windows; out_proj(w) is
    interleaved into window w+1's kt loop so the exp engines never stall
    behind it; only out_proj(last) remains as a tail.
  - input DMAs issue from sync/gpsimd only; Scalar stays clear for exp.

Host sums the 4 group partials per batch, transposes back, and adds the
exact bias corrections: b_out plus w_out @ b_v.
"""

import math

import numpy as np

import concourse.tile as tile
from concourse import bacc, mybir
from concourse.bass_utils import run_bass_kernel_spmd

B, S, H = 2, 2048, 1024
NH, HD = 16, 64
NCORES = 8
NGROUP = 4              # head groups = cores per batch
HPG = NH // NGROUP      # 4 heads per group
DG = HPG * HD           # 256 channels per group
P = 128
SCALE = float(HD) ** -0.5

FP32 = mybir.dt.float32
BF16 = mybir.dt.bfloat16
I16 = mybir.dt.int16

S_TILES = S // P        # 16 key/token tiles
HC = H // P             # 8 contraction chunks over H
QKR = 2 * DG            # 512 q+k rows
QKC = QKR // P          # 4 chunks of qk rows
QT = 1024               # scores tile: 2 heads x 512 q
WIN = 512               # q window
NWIN = S // WIN         # 4

A16 = 128.0 / math.log(2.0)
B16 = 16250.5           # Schraudolph bias (RNE f32->i16, bf16 bit pattern)

# kt tiles routed to the DVE Schraudolph exp (per window); keep small:
# each tile carries ~2% rms error vs ACT's exact exp.
DVE_KTS = (2, 5, 8, 11, 14)

_NC_CACHE = {}
LAST_RESULT = None      # BassKernelResults of the most recent run (for test.py)


def _body(tc, x_t, wqk_t, wv_t, wo_t, act_bias, mask_info, bias_zero, out_t):
    nc = tc.nc
    mask_uniform, dve_s2_imm, no_dve = mask_info
    with (
        tc.tile_pool(name="const", bufs=1) as const,
        tc.tile_pool(name="big", bufs=1) as big,
        tc.tile_pool(name="pt_pool", bufs=4) as pt_pool,
        tc.tile_pool(name="rz_pool", bufs=2) as rz_pool,
        tc.tile_pool(name="osb_pool", bufs=2) as osb_pool,
        tc.tile_pool(name="stp", bufs=2, space="PSUM") as stp,
        tc.tile_pool(name="avps", bufs=3, space="PSUM") as avps,
        tc.tile_pool(name="iop", bufs=1, space="PSUM") as iop,
    ):
        # ---------- PE warm-up: keep HAM busy during the input DMA ----------
        wu_sb = const.tile([P, 128], BF16, name="wu_sb")
        nc.vector.memset(wu_sb[:], 0.0)
        wu_ps = iop.tile([P, 128], FP32, name="wu_ps", tag="io")
        for _ in range(36):
            nc.tensor.matmul(wu_ps[:], lhsT=wu_sb[:], rhs=wu_sb[:],
                             start=True, stop=True)

        # ---------- input DMAs, ordered by first use ----------
        x_sb = big.tile([P, HC, S], BF16, name="x_sb")
        x_r = x_t.rearrange("(c p) s -> p c s", p=P)
        wqk_sb = const.tile([P, HC, QKR], BF16, name="wqk_sb")
        wqk_r = wqk_t.rearrange("(c p) r -> p c r", p=P)
        nc.sync.dma_start(wqk_sb[:], wqk_r[:])
        nc.gpsimd.dma_start(x_sb[:, 0:4, 0:WIN], x_r[:, 0:4, 0:WIN])
        nc.scalar.dma_start(x_sb[:, 4:8, 0:WIN], x_r[:, 4:8, 0:WIN])
        nc.gpsimd.dma_start(x_sb[:, :, WIN:2 * WIN], x_r[:, :, WIN:2 * WIN])
        nc.sync.dma_start(x_sb[:, :, 2 * WIN:3 * WIN], x_r[:, :, 2 * WIN:3 * WIN])
        nc.gpsimd.dma_start(x_sb[:, :, 3 * WIN:S], x_r[:, :, 3 * WIN:S])
        wv_sb = const.tile([P, HC, DG], BF16, name="wv_sb")
        nc.gpsimd.dma_start(wv_sb[:], wv_t.rearrange("(c p) r -> p c r", p=P))
        wo_sb = const.tile([P, DG // P, H], BF16, name="wo_sb")
        nc.sync.dma_start(wo_sb[:], wo_t.rearrange("(c p) r -> p c r", p=P))
        ab_sb = const.tile([P, S_TILES], FP32, name="ab_sb")
        nc.gpsimd.dma_start(ab_sb[:], act_bias)
        if not mask_uniform:
            db_sb = const.tile([P, S_TILES], FP32, name="db_sb")
            nc.gpsimd.dma_start(db_sb[:], tc.extra["dve_bias"])
        if not bias_zero:
            bqk_sb = const.tile([P, QKC], FP32, name="bqk_sb")
            nc.gpsimd.dma_start(bqk_sb[:], tc.extra["bqk"])

        qk_sb = big.tile([P, QKC, S], BF16, name="qk_sb")
        # v_aug per (tile, head): [v (64 cols) | ones (64 cols)]
        v_sb = big.tile([P, S_TILES, HPG, 2 * HD], BF16, name="v_sb")
        attn_sb = big.tile([P, DG // P, S], BF16, name="attn_sb")

        ones_sb = const.tile([P, HPG, HD], BF16, name="ones_sb")
        nc.vector.memset(ones_sb[:], 1.0)
        for tt in range(S_TILES):
            nc.vector.tensor_copy(v_sb[:, tt, :, HD:2 * HD], ones_sb[:])

        # ---------- qkv projection ----------
        def qk_group(rc, i, pool):
            qk_ps = pool.tile([P, 512], FP32, name="qk_ps",
                              tag="av" if pool is avps else "io")
            for c in range(HC):
                nc.tensor.matmul(
                    qk_ps[:],
                    lhsT=wqk_sb[:, c, rc * P:(rc + 1) * P],
                    rhs=x_sb[:, c, i * WIN:(i + 1) * WIN],
                    start=(c == 0), stop=(c == HC - 1),
                )
            dst = qk_sb[:, rc, i * WIN:(i + 1) * WIN]
            if bias_zero:
                nc.vector.tensor_copy(dst, qk_ps[:])
            else:
                nc.vector.tensor_scalar_add(dst, qk_ps[:], bqk_sb[:, rc:rc + 1])

        def v_group(tp):
            # token tiles (2tp, 2tp+1) side by side in one psum slot
            v_ps = avps.tile([P, 512], FP32, name="v_ps", tag="av")
            for half in range(2):
                tt = 2 * tp + half
                for c in range(HC):
                    nc.tensor.matmul(
                        v_ps[:, half * DG:(half + 1) * DG],
                        lhsT=x_sb[:, c, tt * P:(tt + 1) * P],
                        rhs=wv_sb[:, c, :],
                        start=(c == 0), stop=(c == HC - 1),
                    )
            nc.vector.tensor_copy(
                v_sb[:, 2 * tp:2 * tp + 2, :, 0:HD],
                v_ps[:].rearrange("p (t h d) -> p t h d", t=2, d=HD),
            )

        # k chunks + v first (attention needs every token of k/v), then q(0)
        qk_group(2, 0, avps)
        for i in range(1, NWIN):
            qk_group(2, i, avps)
            v_group(2 * (i - 1))
            v_group(2 * (i - 1) + 1)
        v_group(6)
        v_group(7)
        qk_group(0, 0, avps)

        # ---------- attention ----------
        o_r = out_t.rearrange("(c p) s -> p c s", p=P)
        dve_s1 = SCALE * A16

        def out_proj_chunks(w, jjs, o_sb):
            qlo = w * WIN
            for jj in jjs:
                o_ps = iop.tile([P, 512], FP32, name="o_ps", tag="io")
                for kc in range(DG // P):
                    nc.tensor.matmul(
                        o_ps[:],
                        lhsT=wo_sb[:, kc, jj * P:(jj + 1) * P],
                        rhs=attn_sb[:, kc, qlo:qlo + WIN],
                        start=(kc == 0), stop=(kc == DG // P - 1),
                    )
                nc.vector.tensor_copy(o_sb[:, jj, :], o_ps[:])

        def attn_window(qc, w, hooks):
            """One 512-q window for head pair qc. hooks: {kt: [fn, ...]}"""
            qlo = w * WIN
            av = [avps.tile([P, 512], FP32, name=f"av{h}", tag="av")
                  for h in range(2)]
            def av_mms(kt, pt):
                for h in range(2):
                    nc.tensor.matmul(
                        av[h][:],
                        lhsT=v_sb[:, kt, 2 * qc + h, :],
                        rhs=pt[:, h * 512:(h + 1) * 512],
                        start=(kt == 0), stop=(kt == S_TILES - 1),
                    )

            pts = {}
            for kt in range(S_TILES):
                st = stp.tile([P, QT], FP32, name="st", tag="st")
                for half in range(2):
                    off = half * HD
                    nc.tensor.matmul(
                        st[:, half * 512:(half + 1) * 512],
                        lhsT=qk_sb[off:off + HD, 2 + qc, kt * P:(kt + 1) * P],
                        rhs=qk_sb[off:off + HD, qc, qlo:qlo + WIN],
                        start=True, stop=True,
                    )
                pt = pt_pool.tile([P, QT], BF16, name="pt", tag="pt")
                pts[kt] = pt
                if kt in DVE_KTS and not no_dve:
                    s2 = dve_s2_imm if mask_uniform else db_sb[:, kt:kt + 1]
                    nc.vector.tensor_scalar(
                        pt[:].bitcast(I16), st[:],
                        dve_s1, s2,
                        mybir.AluOpType.mult, mybir.AluOpType.add,
                    )
                else:
                    nc.scalar.activation(
                        pt[:], st[:],
                        mybir.ActivationFunctionType.Exp,
                        bias=ab_sb[:, kt:kt + 1],
                        scale=SCALE,
                    )
                if kt >= 1:
                    av_mms(kt - 1, pts.pop(kt - 1))
                for fn in hooks.get(kt, ()):
                    fn()
            av_mms(S_TILES - 1, pts.pop(S_TILES - 1))
            zc = rz_pool.tile([HD, QT], FP32, name="zc", tag="zc")
            for h in range(2):
                nc.vector.tensor_copy(
                    zc[:, h * 512:(h + 1) * 512], av[h][HD:2 * HD, :])
            rz = rz_pool.tile([HD, QT], FP32, name="rz", tag="rz")
            nc.vector.reciprocal_approx_fast(rz[:], zc[:])
            for h in range(2):
                nc.vector.tensor_mul(
                    attn_sb[h * HD:(h + 1) * HD, qc, qlo:qlo + WIN],
                    av[h][0:HD, :],
                    rz[:, h * 512:(h + 1) * 512],
                )

        # pair A windows: interleave pair B's projection (+ pair A's next q)
        for w in range(NWIN):
            hooks = {3: [lambda w=w: qk_group(1, w, iop)],
                     9: [lambda w=w: qk_group(3, w, iop)]}
            if w < NWIN - 1:
                hooks[13] = [lambda w=w: qk_group(0, w + 1, iop)]
            attn_window(0, w, hooks)

        # pair B windows: interleave out-proj of the previous window
        for w in range(NWIN):
            hooks = {}
            if w > 0:
                pw = w - 1
                o_sb = osb_pool.tile([P, HC, 512], BF16, name="o_sb", tag="osb")
                hooks = {
                    2: [lambda pw=pw, o=o_sb: out_proj_chunks(pw, (0, 1), o)],
                    6: [lambda pw=pw, o=o_sb: out_proj_chunks(pw, (2, 3), o)],
                    10: [lambda pw=pw, o=o_sb: out_proj_chunks(pw, (4, 5), o)],
                    14: [lambda pw=pw, o=o_sb: out_proj_chunks(pw, (6, 7), o),
                         lambda pw=pw, o=o_sb: nc.sync.dma_start(
                             o_r[:, :, pw * WIN:(pw + 1) * WIN], o[:])],
                }
            attn_window(1, w, hooks)
        # tail: last window's out-proj, DMA pipelined per 2 chunks
        o_sb = osb_pool.tile([P, HC, 512], BF16, name="o_sb", tag="osb")
        qlo = (NWIN - 1) * WIN
        for j0 in range(0, HC, 2):
            out_proj_chunks(NWIN - 1, (j0, j0 + 1), o_sb)
            nc.sync.dma_start(o_r[:, j0:j0 + 2, qlo:S], o_sb[:, j0:j0 + 2, :])


def _build(mask_uniform, dve_s2_imm, bias_zero, no_dve):
    nc = bacc.Bacc(
        "TRN2",
        target_bir_lowering=False,
        debug=False,
        enable_asserts=True,
        num_devices=NCORES,
    )
    x_t = nc.dram_tensor("x_t", [H, S], BF16, kind="ExternalInput").ap()
    wqk_t = nc.dram_tensor("wqk_t", [H, QKR], BF16, kind="ExternalInput").ap()
    wv_t = nc.dram_tensor("wv_t", [H, DG], BF16, kind="ExternalInput").ap()
    wo_t = nc.dram_tensor("wo_t", [DG, H], BF16, kind="ExternalInput").ap()
    act_bias = nc.dram_tensor("act_bias", [P, S_TILES], FP32,
                              kind="ExternalInput").ap()
    out_t = nc.dram_tensor("out_t", [H, S], BF16, kind="ExternalOutput").ap()

    with tile.TileContext(nc) as tc:
        tc.extra = {}
        if not mask_uniform:
            tc.extra["dve_bias"] = nc.dram_tensor(
                "dve_bias", [P, S_TILES], FP32, kind="ExternalInput").ap()
        if not bias_zero:
            tc.extra["bqk"] = nc.dram_tensor(
                "bqk", [P, QKC], FP32, kind="ExternalInput").ap()
        _body(tc, x_t, wqk_t, wv_t, wo_t, act_bias,
              (mask_uniform, dve_s2_imm, no_dve), bias_zero, out_t)
    nc.compile()
    return nc


def _get_nc(mask_uniform=True, dve_s2_imm=None, bias_zero=True):
    import os
    if dve_s2_imm is None:
        dve_s2_imm = B16
    no_dve = bool(int(os.environ.get("KERNEL_NO_DVE_EXP", "0")))
    key = (mask_uniform, dve_s2_imm, bias_zero, no_dve)
    if key not in _NC_CACHE:
        _NC_CACHE[key] = _build(mask_uniform, dve_s2_imm, bias_zero, no_dve)
    return _NC_CACHE[key]


def _variant(attention_mask, b_qkv):
    m0 = float(np.asarray(attention_mask).reshape(-1)[0])
    mask_uniform = bool(np.all(attention_mask == m0))
    dve_s2_imm = (B16 + A16 * m0) if mask_uniform else None
    bias_zero = bool(np.all(b_qkv == 0.0))
    return mask_uniform, dve_s2_imm, bias_zero


def make_in_maps(hidden_states, attention_mask, w_qkv, b_qkv, w_out):
    import ml_dtypes

    bf16 = ml_dtypes.bfloat16
    mask_uniform, _, bias_zero = _variant(attention_mask, b_qkv)
    in_maps = []
    for core in range(NCORES):
        b, g = divmod(core, NGROUP)
        wq = w_qkv[0 * H + g * DG:0 * H + (g + 1) * DG]
        wk = w_qkv[1 * H + g * DG:1 * H + (g + 1) * DG]
        wv = w_qkv[2 * H + g * DG:2 * H + (g + 1) * DG]
        mask = np.asarray(attention_mask[b], np.float32)
        ab = mask.reshape(S_TILES, P).T              # [P, S_TILES]
        m = {
            "x_t": np.ascontiguousarray(hidden_states[b].T).astype(bf16),
            "wqk_t": np.ascontiguousarray(
                np.concatenate([wq, wk], 0).T).astype(bf16),
            "wv_t": np.ascontiguousarray(wv.T).astype(bf16),
            "wo_t": np.ascontiguousarray(
                w_out[:, g * DG:(g + 1) * DG].T).astype(bf16),
            "act_bias": np.ascontiguousarray(ab, dtype=np.float32),
        }
        if not mask_uniform:
            m["dve_bias"] = np.ascontiguousarray(
                A16 * ab + B16, dtype=np.float32)
        if not bias_zero:
            m["bqk"] = np.ascontiguousarray(
                np.concatenate([b_qkv[g * DG:(g + 1) * DG],
                                b_qkv[H + g * DG:H + (g + 1) * DG]])
                .reshape(QKC, P).T, dtype=np.float32)
        in_maps.append(m)
    return in_maps


def kernel(hidden_states, attention_mask, w_qkv, b_qkv, w_out, b_out):
    global LAST_RESULT
    hidden_states = np.asarray(hidden_states, dtype=np.float32)
    attention_mask = np.asarray(attention_mask, dtype=np.float32)
    w_qkv = np.asarray(w_qkv, dtype=np.float32)
    b_qkv = np.asarray(b_qkv, dtype=np.float32)
    w_out = np.asarray(w_out, dtype=np.float32)
    b_out = np.asarray(b_out, dtype=np.float32)

    mask_uniform, dve_s2_imm, bias_zero = _variant(attention_mask, b_qkv)
    nc = _get_nc(mask_uniform, dve_s2_imm, bias_zero)
    in_maps = make_in_maps(hidden_states, attention_mask, w_qkv, b_qkv, w_out)

    import os
    trace = bool(int(os.environ.get("KERNEL_TRACE", "0")))
    res = run_bass_kernel_spmd(
        nc, in_maps, core_ids=list(range(NCORES)), trace=trace,
    )
    LAST_RESULT = res

    out = np.zeros((B, S, H), np.float32)
    vbias = w_out @ b_qkv[2 * H:]          # exact v-bias correction
    for b in range(B):
        acc = res.results[b * NGROUP + 0]["out_t"].astype(np.float32)
        for g in range(1, NGROUP):
            acc = acc + res.results[b * NGROUP + g]["out_t"].astype(np.float32)
        out[b] = acc.T + b_out + vbias
    return out
